# revision 38
# baseline (speedup 1.0000x reference)
"""Trainium2 Bass kernel for BasicLSTM (nn_BasicLSTM_16320875724833).

Problem: inputs [256, 1024, 128] f32; LSTM(H=256) over T=1024 steps, then
linear [256->2] + softmax on the final hidden state. Output [256, 2] f32.

Strategy (8 cores, data-parallel over batch, 32 rows/core):
  - All state kept "transposed" (feature-major): hT/cT are [128p, 2, 32].
  - fp8e4 DoubleRow matmuls everywhere on the recurrence path:
      * recurrence: one MM per gate chunk (K=256 folded into the pair dim),
        8 MMs/step, rhs = hT [128, 2, 32] fp8.
      * input projection: x is pre-transposed on host to [128, 2, T*BLOC]
        fp8 where pair-half 1 is a (d==0) indicator channel; the projection
        weights' pair-half 1 carries the bias row, so one DR MM per gate
        chunk computes W_ih^T x + (b_ih + b_hh) for 4 steps at once --
        no separate bias matmuls.
  - Gate chunk order (host-side row permutation of the PyTorch [i,f,g,o]
    layout): [g0,g1, i0,i1, f0,f1, o0,o1] so tanh(g) fires after 2 MMs and
    sigmoid(i,f) after 6.
  - Head: softmax over 2 classes == [sigmoid(d), sigmoid(-d)] with
    d = h @ (W_lin[0]-W_lin[1]) + (b_lin[0]-b_lin[1]); h rebuilt in f32
    from the final step's sigmoid(o) and tanh(c) so fp8 h never feeds it.
"""

import numpy as np

# ---- problem constants (hardcoded; kernel.py must be self-contained) ----
B, T, D, H = 256, 1024, 128, 256
NCORES = 8
BLOC = B // NCORES          # 32 batch rows per core
GC = 8                      # gate chunks of 128 (4H = 1024)
KC = 2                      # hidden chunks of 128 (H = 256)
G4 = 4                      # timesteps per PSUM group
TC = 64                     # time chunk for x layout (keeps the DoubleRow
                            # pair-dim stride = TC*BLOC = 2048 within the
                            # ISA's 16-bit AP step field)

import os
SIGSPLIT = os.environ.get("K_SIGSPLIT", "1") == "1"  # (i,f) + (o) split
GBUFS = int(os.environ.get("K_GBUFS", "2"))          # gates PSUM pool buffers
SIGTRICK = os.environ.get("K_SIG", "1") == "1"       # sigmoid-only EW chain
PACK10 = os.environ.get("K_PACK10", "0") == "1"      # tanh tables + 10-slot state
POOLFC = os.environ.get("K_POOL", "0") == "1"        # f*c multiply on Pool
NOCUST = os.environ.get("K_NOCUST", "0") == "1"      # avoid custom DVE ops
NODR = os.environ.get("K_NODR", "0") == "1"          # plain (non-DoubleRow) rec MMs
FORI = os.environ.get("K_FORI", "1") == "1"          # hardware loop over TC chunks
ABL = os.environ.get("K_ABL", "")    # timing-only ablations, comma-separated
REPEAT = 1                  # timing-only: run the recurrence REPEAT times

_cache = {}


def _build_program(seq_len=T):
    import concourse.bass as bass
    import concourse.mybir as mybir
    from concourse import bacc
    from concourse.tile import TileContext
    from contextlib import ExitStack

    f16 = mybir.dt.float16
    f32 = mybir.dt.float32
    f8 = mybir.dt.float8e4
    AF = mybir.ActivationFunctionType
    DR = mybir.MatmulPerfMode.DoubleRow

    nc = bacc.Bacc(None, target_bir_lowering=False)

    # x pre-transposed+interleaved on host:
    # [128(d), seq/TC, 2(pair), TC*BLOC] fp8; pair-half 1 = (d==0) indicator
    ntc = (seq_len + TC - 1) // TC
    x = nc.dram_tensor("x", [128, ntc, 2, TC * BLOC], f8, kind="ExternalInput")
    # projection weights with bias folded into pair-half 1: [128, 2, 4H] fp8
    wih = nc.dram_tensor("wih", [128, 2, 4 * H], f8, kind="ExternalInput")
    # recurrence weights: [128(k), KC(pair), 4H] fp8
    whh = nc.dram_tensor("whh", [128, KC, 4 * H], f8, kind="ExternalInput")
    wd = nc.dram_tensor("wd", [128, KC, 1], f32, kind="ExternalInput")
    out = nc.dram_tensor("out", [1, 2, BLOC], f32, kind="ExternalOutput")

    with ExitStack() as ctx:
        tc = ctx.enter_context(TileContext(nc))
        consts = ctx.enter_context(tc.tile_pool(name="consts", bufs=1))
        state = ctx.enter_context(tc.tile_pool(name="state", bufs=1))
        xbp = ctx.enter_context(tc.tile_pool(name="xbp", bufs=1))
        ew = ctx.enter_context(tc.tile_pool(name="ew", bufs=3))
        gpsum = ctx.enter_context(tc.tile_pool(name="gpsum", bufs=1, space="PSUM"))
        hpsum = ctx.enter_context(tc.tile_pool(name="hpsum", bufs=1, space="PSUM"))
        # manual PSUM double-buffer (static addresses; safe inside For_i)
        pbuf = [gpsum.tile([128, GC, G4, BLOC], f32, name=f"P{j}")
                for j in range(2)]

        # constants into SBUF
        wih_sb = consts.tile([128, 2, 4 * H], f8)
        nc.sync.dma_start(out=wih_sb[:, :, :], in_=wih[:, :, :])
        whh_sb = consts.tile([128, KC, 4 * H], f8)
        nc.sync.dma_start(out=whh_sb[:, :, :], in_=whh[:, :, :])
        wd_sb = consts.tile([128, KC, 1], f32)
        nc.sync.dma_start(out=wd_sb[:, :, :], in_=wd[:, :, :])

        # double-buffered by step parity to keep cross-step WAR hazards off
        # the critical path: at the start of step s, h(s-1) lives in
        # hbuf[s%2]; mul_h(s) writes h(s) into hbuf[(s+1)%2]. gcat packs
        # [ghat(2 chunks), c(2 chunks)] so one tensor_mul computes both
        # i*ghat and f*c; tanh(g)(s) writes gbuf[s%2][0:2] while add(s)
        # writes c(s) into gbuf[(s+1)%2][2:4].
        hbuf = [state.tile([128, KC, BLOC], f8, name=f"hT{j}") for j in range(2)]
        gbuf = [state.tile([128, 4, BLOC], f32, name=f"gcat{j}") for j in range(2)]
        # PACK10 state: slots [ghat0,ghat1, si0,si1, sf0,sf1, so0,so1, c0,c1]
        sbuf10 = [state.tile([128, 10, BLOC], f32, name=f"s10_{j}") for j in range(2)]
        for j in range(2):
            nc.vector.memset(hbuf[j][:, :, :], 0.0)
            nc.vector.memset(gbuf[j][:, :, :], 0.0)
            nc.vector.memset(sbuf10[j][:, :, :], 0.0)

        if not FORI:
            # whole input (already d-major / pair-interleaved)
            xTb = xbp.tile([128, ntc, 2, TC * BLOC], f8)
            nc.sync.dma_start(out=xTb[:, :, :, :], in_=x[:, :, :, :])
        else:
            assert ntc % 2 == 0, "FORI needs an even number of x chunks"

        def emit_group(xsrc, tau0, sbase, P):
            # one PSUM group: projection+bias for G4 steps, then the G4
            # recurrent steps. xsrc is a static [128, 2, TC*BLOC] view/tile;
            # sbase is the python step index (parity source) within the
            # unrolled region; P a static PSUM tile.
            xvw = xsrc[:, :, tau0 * BLOC:(tau0 + G4) * BLOC]
            for gc in range(GC):
                # start=True zeroes the whole 2KB PSUM bank (4 gate chunks),
                # so only the first MM touching each bank may set it
                nc.tensor.matmul(
                    P[:, gc, :, :].rearrange("p t b -> p (t b)"),
                    lhsT=wih_sb[:, :, gc * 128:(gc + 1) * 128],
                    rhs=xvw,
                    start=(gc % 4 == 0), stop=False, skip_group_check=True,
                    perf_mode=DR,
                )
            for tt in range(G4):
                s = sbase + tt
                hT = hbuf[s % 2]
                hTn = hbuf[(s + 1) % 2]
                gcat = gbuf[s % 2]
                gcatn = gbuf[(s + 1) % 2]
                # recurrence: one DoubleRow MM per gate chunk (K=256)
                if NODR:
                    for gc in range(GC):
                        for kc in range(KC):
                            nc.tensor.matmul(
                                P[:, gc, tt, :],
                                lhsT=whh_sb[:, kc, gc * 128:(gc + 1) * 128],
                                rhs=hT[:, kc, :],
                                start=False, stop=(kc == KC - 1),
                                skip_group_check=True,
                            )
                else:
                    for gc in range(GC):
                        nc.tensor.matmul(
                            P[:, gc, tt, :],
                            lhsT=whh_sb[:, :, gc * 128:(gc + 1) * 128],
                            rhs=hT[:, :, :],
                            start=False, stop=True,
                            skip_group_check=True,
                            perf_mode=DR,
                        )
                abl = ABL.split(",")
                if PACK10:
                    # tanh tables; one strided mul computes [ghat,sf]*[si,c]
                    cur = sbuf10[s % 2]
                    nxt = sbuf10[(s + 1) % 2]
                    nc.scalar.activation(cur[:, 0:2, :], P[:, 0:2, tt, :], AF.Tanh)
                    nc.scalar.activation(cur[:, 2:8, :], P[:, 2:8, tt, :], AF.Sigmoid)
                    prod = ew.tile([128, 4, BLOC], f32, tag="prod")
                    # [ghat, sf] * [si, c] = slots {0,1,4,5} * {2,3,8,9}
                    in0 = cur[:, 0:6, :].rearrange(
                        "p (a b) x -> p a b x", a=3)[:, 0::2, :, :]
                    in1 = cur[:, 2:10, :].rearrange(
                        "p (a b) x -> p a b x", a=4)[:, 0::3, :, :]
                    nc.vector.tensor_mul(
                        prod[:, :, :].rearrange("p (a b) x -> p a b x", a=2),
                        in0, in1)
                    nc.vector.tensor_add(nxt[:, 8:10, :], prod[:, 0:2, :],
                                         prod[:, 2:4, :])
                    thc = ew.tile([128, 2, BLOC], f32, tag="thc")
                    nc.scalar.activation(thc[:, :, :], nxt[:, 8:10, :], AF.Tanh)
                    sb_ifo = cur  # head reads sigma(o) at [6:8]
                    nc.vector.tensor_mul(hTn[:, :, :], cur[:, 6:8, :], thc[:, :, :])
                elif SIGTRICK:
                    # sigmoid-only chain (g rows pre-scaled 2x on host):
                    #   s = sigmoid([2g, i, f, o])
                    #   ig = (2*s_g - 1) * s_i        (tanh(g) fused into mul)
                    #   fc = s_f * c
                    #   c' = ig + fc
                    #   h  = (2*sigmoid(2c') - 1) * s_o
                    from concourse.dve_ops import AFFINE_MUL_REDUCE
                    sb_sig = ew.tile([128, 8, BLOC], f32, tag="sb_sig")
                    if SIGSPLIT:
                        nc.scalar.activation(sb_sig[:, 0:6, :], P[:, 0:6, tt, :],
                                             AF.Sigmoid)
                        nc.scalar.activation(sb_sig[:, 6:8, :], P[:, 6:8, tt, :],
                                             AF.Sigmoid)
                    else:
                        nc.scalar.activation(sb_sig[:, :, :], P[:, 0:8, tt, :],
                                             AF.Sigmoid)
                    prod = ew.tile([128, 4, BLOC], f32, tag="prod")
                    import concourse.mybir as _mb
                    if NOCUST:
                        ghat = ew.tile([128, 2, BLOC], f32, tag="ghat")
                        nc.vector.tensor_scalar(
                            ghat[:, :, :], sb_sig[:, 0:2, :], 2.0, -1.0,
                            _mb.AluOpType.mult, _mb.AluOpType.add)
                        nc.vector.tensor_mul(prod[:, 0:2, :], ghat[:, :, :],
                                             sb_sig[:, 2:4, :])
                    else:
                        nc.vector._custom_dve(
                            AFFINE_MUL_REDUCE, out=prod[:, 0:2, :],
                            in0=sb_sig[:, 0:2, :], in1=sb_sig[:, 2:4, :],
                            s0=2.0, s1=-1.0)
                    eng_fc = nc.gpsimd if POOLFC else nc.vector
                    eng_fc.tensor_mul(prod[:, 2:4, :], sb_sig[:, 4:6, :],
                                      gcat[:, 2:4, :])
                    nc.vector.tensor_add(gcatn[:, 2:4, :], prod[:, 0:2, :],
                                         prod[:, 2:4, :])
                    thc = ew.tile([128, 2, BLOC], f32, tag="thc")
                    nc.scalar.activation(thc[:, :, :], gcatn[:, 2:4, :],
                                         AF.Sigmoid, scale=2.0)
                    sb_ifo = sb_sig  # head reads sigma(o) at [6:8]
                    if NOCUST:
                        th2 = ew.tile([128, 2, BLOC], f32, tag="th2")
                        nc.vector.tensor_scalar(
                            th2[:, :, :], thc[:, :, :], 2.0, -1.0,
                            _mb.AluOpType.mult, _mb.AluOpType.add)
                        nc.vector.tensor_mul(hTn[:, :, :], th2[:, :, :],
                                             sb_sig[:, 6:8, :])
                    else:
                        nc.vector._custom_dve(
                            AFFINE_MUL_REDUCE, out=hTn[:, :, :],
                            in0=thc[:, :, :], in1=sb_sig[:, 6:8, :],
                            s0=2.0, s1=-1.0)
                else:
                    # elementwise cell update:
                    #   ghat = tanh(g); [i,f,o] = sigmoid(...)
                    #   prod = [i, f] * [ghat, c];  c = prod0 + prod1
                    #   h = o * tanh(c)
                    if "tg" not in abl:
                        nc.scalar.activation(gcat[:, 0:2, :], P[:, 0:2, tt, :], AF.Tanh)
                    sb_ifo = ew.tile([128, 6, BLOC], f32, tag="sb_ifo")
                    if "sif" not in abl:
                        if SIGSPLIT:
                            nc.scalar.activation(sb_ifo[:, 0:4, :], P[:, 2:6, tt, :], AF.Sigmoid)
                            nc.scalar.activation(sb_ifo[:, 4:6, :], P[:, 6:8, tt, :], AF.Sigmoid)
                        else:
                            nc.scalar.activation(sb_ifo[:, :, :], P[:, 2:8, tt, :], AF.Sigmoid)
                    prod = ew.tile([128, 4, BLOC], f32, tag="prod")
                    if "mul" not in abl:
                        nc.vector.tensor_mul(prod[:, :, :], sb_ifo[:, 0:4, :], gcat[:, :, :])
                    if "add" not in abl:
                        nc.vector.tensor_add(gcatn[:, 2:4, :], prod[:, 0:2, :], prod[:, 2:4, :])
                    thc = ew.tile([128, 2, BLOC], f32, tag="thc")
                    if "tc" not in abl:
                        nc.scalar.activation(thc[:, :, :], gcatn[:, 2:4, :], AF.Tanh)
                    if "mh" not in abl:
                        nc.vector.tensor_mul(hTn[:, :, :], sb_ifo[:, 4:6, :], thc[:, :, :])
            return sb_ifo, thc

        if FORI:
            # stage x chunks into static SBUF tiles via (dynamic-offset) DMA;
            # two chunks per iteration so buffer choice stays python-static
            xst = [xbp.tile([128, 2, TC * BLOC], f8, name=f"xst{j}")
                   for j in range(2)]
            gpc = TC // G4          # groups per chunk
            for _ in range(REPEAT):
                with tc.For_i(0, ntc, step=2) as ci_var:
                    nc.sync.dma_start(out=xst[0][:, :, :], in_=x[:, ci_var, :, :])
                    nc.sync.dma_start(out=xst[1][:, :, :],
                                      in_=x[:, ci_var + 1, :, :])
                    for half in range(2):
                        for gl in range(gpc):
                            g = half * gpc + gl
                            sb_ifo, thc = emit_group(
                                xst[half], gl * G4, g * G4, pbuf[g % 2])
        else:
            for gi in range(REPEAT * seq_len // G4):
                t0 = (gi * G4) % seq_len
                sb_ifo, thc = emit_group(
                    xTb[:, t0 // TC, :, :], t0 % TC, gi * G4, pbuf[gi % 2])

        # head: rebuild final h in f32 (avoid fp8 h), then
        # d = h @ w_d; probs = [sigmoid(d+bd), sigmoid(-d-bd)]
        hT32 = consts.tile([128, KC, BLOC], f32)
        if PACK10:
            nc.vector.tensor_mul(hT32[:, :, :], sb_ifo[:, 6:8, :], thc[:, :, :])
        elif SIGTRICK:
            if NOCUST:
                import concourse.mybir as _mb
                th2h = consts.tile([128, KC, BLOC], f32)
                nc.vector.tensor_scalar(
                    th2h[:, :, :], thc[:, :, :], 2.0, -1.0,
                    _mb.AluOpType.mult, _mb.AluOpType.add)
                nc.vector.tensor_mul(hT32[:, :, :], th2h[:, :, :],
                                     sb_ifo[:, 6:8, :])
            else:
                from concourse.dve_ops import AFFINE_MUL_REDUCE
                nc.vector._custom_dve(
                    AFFINE_MUL_REDUCE, out=hT32[:, :, :], in0=thc[:, :, :],
                    in1=sb_ifo[:, 6:8, :], s0=2.0, s1=-1.0)
        else:
            nc.vector.tensor_mul(hT32[:, :, :], sb_ifo[:, 4:6, :], thc[:, :, :])
        hps = hpsum.tile([1, BLOC], f32)
        nc.tensor.matmul(hps[:, :], lhsT=wd_sb[:, 0, :], rhs=hT32[:, 0, :],
                         start=True, stop=False, skip_group_check=True)
        nc.tensor.matmul(hps[:, :], lhsT=wd_sb[:, 1, :], rhs=hT32[:, 1, :],
                         start=False, stop=True, skip_group_check=True)
        outsb = consts.tile([1, 2, BLOC], f32)
        bd_pos = consts.tile([1, 1], f32)
        bd_neg = consts.tile([1, 1], f32)
        nc.vector.memset(bd_pos[:, :], float(_cache["b_d"]))
        nc.vector.memset(bd_neg[:, :], -float(_cache["b_d"]))
        nc.scalar.activation(outsb[:, 0, :], hps[:, :], AF.Sigmoid,
                             bias=bd_pos[:, :], scale=1.0)
        nc.scalar.activation(outsb[:, 1, :], hps[:, :], AF.Sigmoid,
                             bias=bd_neg[:, :], scale=-1.0)
        nc.sync.dma_start(out=out[:, :, :], in_=outsb[:, :, :])

    nc.compile()
    return nc


def _prep_host(inputs, W_ih, W_hh, b_ih, b_hh, W_lin, b_lin):
    """Host-side weight preprocessing: gate permutation + transposed layouts."""
    import concourse.mybir as _mb
    f8np = _mb.dt.np(_mb.dt.float8e4)
    # PyTorch gate row order [i, f, g, o] (256 each) -> chunk order
    # [g0, g1, i0, i1, f0, f1, o0, o1] (128-row chunks)
    perm = np.concatenate([
        np.arange(512, 768),    # g
        np.arange(0, 256),      # i
        np.arange(256, 512),    # f
        np.arange(768, 1024),   # o
    ])

    Wih_p = np.ascontiguousarray(W_ih[perm]).astype(np.float32)  # [1024, 128]
    Whh_p = np.ascontiguousarray(W_hh[perm]).astype(np.float32)  # [1024, 256]
    b_p = (b_ih + b_hh)[perm].astype(np.float32)        # [1024]
    if SIGTRICK and not PACK10:
        # tanh(g) = 2*sigmoid(2g) - 1: fold the 2x into the g-gate rows
        # (exact power-of-2 scale, no extra fp8 rounding error)
        Wih_p[0:256] *= 2.0
        Whh_p[0:256] *= 2.0
        b_p[0:256] *= 2.0

    # projection lhsT with bias in pair-half 1: [128(d), 2, 1024]
    wih_host = np.zeros((128, 2, 4 * H), np.float32)
    wih_host[:, 0, :] = Wih_p.T
    wih_host[0, 1, :] = b_p
    wih_host = wih_host.astype(f8np)

    # recurrence lhsT: [128(k within chunk), KC, 1024]
    whh_host = np.ascontiguousarray(
        Whh_p.T.reshape(KC, 128, 4 * H).transpose(1, 0, 2)
    ).astype(f8np)

    w_d = (W_lin[0] - W_lin[1]).astype(np.float32)                  # [256]
    wd_host = np.ascontiguousarray(
        w_d.reshape(KC, 128).T.reshape(128, KC, 1)).astype(np.float32)
    b_d = float(b_lin[0] - b_lin[1])

    # x: [256, T, 128] f32 -> [128(d), T/TC, 2(pair), TC, B] fp8 with
    # pair-half 1 = (d==0) indicator (per-core batch slice + reshape to
    # [128, T/TC, 2, TC*BLOC] happens in kernel())
    x8 = inputs.astype(f8np)                                        # [256, T, 128]
    xT = np.transpose(x8, (2, 1, 0))                                # [128, T, 256]
    ntc = T // TC
    x_host = np.zeros((128, ntc, 2, TC, B), f8np)
    x_host[:, :, 0, :, :] = xT.reshape(128, ntc, TC, B)
    x_host[0, :, 1, :, :] = f8np(1.0)
    return x_host, wih_host, whh_host, wd_host, b_d


def kernel(inputs, W_ih, W_hh, b_ih, b_hh, W_lin, b_lin):
    from concourse.bass_utils import run_bass_kernel_spmd

    inputs = np.asarray(inputs, dtype=np.float32)
    x_host, wih_h, whh_h, wd_h, b_d = _prep_host(
        np.asarray(inputs), np.asarray(W_ih), np.asarray(W_hh),
        np.asarray(b_ih), np.asarray(b_hh), np.asarray(W_lin), np.asarray(b_lin))
    if _cache.get("b_d") != b_d or "nc" not in _cache:
        _cache["b_d"] = b_d
        _cache["nc"] = _build_program(T)
    nc = _cache["nc"]

    in_maps = []
    for j in range(NCORES):
        xj = np.ascontiguousarray(x_host[:, :, :, :, j * BLOC:(j + 1) * BLOC])
        in_maps.append({
            "x": xj.reshape(128, T // TC, 2, TC * BLOC),
            "wih": wih_h, "whh": whh_h, "wd": wd_h,
        })

    res = run_bass_kernel_spmd(nc, in_maps, core_ids=list(range(NCORES)))
    _cache["last_result"] = res
    out = np.concatenate(
        [np.asarray(r["out"])[0].T for r in res.results], axis=0)
    return np.ascontiguousarray(out).astype(np.float32)


# revision 44
# speedup vs baseline: 1.0524x; 1.0524x over previous
"""Trainium2 Bass kernel for BasicLSTM (nn_BasicLSTM_16320875724833).

Problem: inputs [256, 1024, 128] f32; LSTM(H=256) over T=1024 steps, then
linear [256->2] + softmax on the final hidden state. Output [256, 2] f32.

Strategy (8 cores, data-parallel over batch, 32 rows/core):
  - All state kept "transposed" (feature-major): hT/cT are [128p, 2, 32].
  - fp8e4 DoubleRow matmuls everywhere on the recurrence path:
      * recurrence: one MM per gate chunk (K=256 folded into the pair dim),
        8 MMs/step, rhs = hT [128, 2, 32] fp8.
      * input projection: x is pre-transposed on host to [128, 2, T*BLOC]
        fp8 where pair-half 1 is a (d==0) indicator channel; the projection
        weights' pair-half 1 carries the bias row, so one DR MM per gate
        chunk computes W_ih^T x + (b_ih + b_hh) for 4 steps at once --
        no separate bias matmuls.
  - Gate chunk order (host-side row permutation of the PyTorch [i,f,g,o]
    layout): [g0,g1, i0,i1, f0,f1, o0,o1] so tanh(g) fires after 2 MMs and
    sigmoid(i,f) after 6.
  - Head: softmax over 2 classes == [sigmoid(d), sigmoid(-d)] with
    d = h @ (W_lin[0]-W_lin[1]) + (b_lin[0]-b_lin[1]); h rebuilt in f32
    from the final step's sigmoid(o) and tanh(c) so fp8 h never feeds it.
"""

import numpy as np

# ---- problem constants (hardcoded; kernel.py must be self-contained) ----
B, T, D, H = 256, 1024, 128, 256
NCORES = 8
BLOC = B // NCORES          # 32 batch rows per core
GC = 8                      # gate chunks of 128 (4H = 1024)
KC = 2                      # hidden chunks of 128 (H = 256)
import os as _os
G4 = int(_os.environ.get("K_G4", "4"))   # timesteps per PSUM group
TC = 64                     # time chunk for x layout (keeps the DoubleRow
                            # pair-dim stride = TC*BLOC = 2048 within the
                            # ISA's 16-bit AP step field)

import os
SIGSPLIT = os.environ.get("K_SIGSPLIT", "1") == "1"  # (i,f) + (o) split
GBUFS = int(os.environ.get("K_GBUFS", "2"))          # gates PSUM pool buffers
SIGTRICK = os.environ.get("K_SIG", "1") == "1"       # sigmoid-only EW chain
PACK10 = os.environ.get("K_PACK10", "0") == "1"      # tanh tables + 10-slot state
POOLFC = os.environ.get("K_POOL", "0") == "1"        # f*c multiply on Pool
NOCUST = os.environ.get("K_NOCUST", "0") == "1"      # avoid custom DVE ops
NODR = os.environ.get("K_NODR", "0") == "1"          # plain (non-DoubleRow) rec MMs
FORI = os.environ.get("K_FORI", "1") == "1"          # hardware loop over TC chunks
F16EW = os.environ.get("K_F16EW", "0") == "1"        # fp16 elementwise temporaries
ABL = os.environ.get("K_ABL", "")    # timing-only ablations, comma-separated
REPEAT = 1                  # timing-only: run the recurrence REPEAT times

_cache = {}


def _build_program(seq_len=T):
    import concourse.bass as bass
    import concourse.mybir as mybir
    from concourse import bacc
    from concourse.tile import TileContext
    from contextlib import ExitStack

    f16 = mybir.dt.float16
    f32 = mybir.dt.float32
    f8 = mybir.dt.float8e4
    AF = mybir.ActivationFunctionType
    DR = mybir.MatmulPerfMode.DoubleRow

    nc = bacc.Bacc(None, target_bir_lowering=False)

    # x pre-transposed+interleaved on host:
    # [128(d), seq/TC, 2(pair), TC*BLOC] fp8; pair-half 1 = (d==0) indicator
    ntc = (seq_len + TC - 1) // TC
    x = nc.dram_tensor("x", [128, ntc, 2, TC * BLOC], f8, kind="ExternalInput")
    # projection weights with bias folded into pair-half 1: [128, 2, 4H] fp8
    wih = nc.dram_tensor("wih", [128, 2, 4 * H], f8, kind="ExternalInput")
    # recurrence weights: [128(k), KC(pair), 4H] fp8
    whh = nc.dram_tensor("whh", [128, KC, 4 * H], f8, kind="ExternalInput")
    wd = nc.dram_tensor("wd", [128, KC, 1], f32, kind="ExternalInput")
    out = nc.dram_tensor("out", [1, 2, BLOC], f32, kind="ExternalOutput")

    with ExitStack() as ctx:
        tc = ctx.enter_context(TileContext(nc))
        consts = ctx.enter_context(tc.tile_pool(name="consts", bufs=1))
        state = ctx.enter_context(tc.tile_pool(name="state", bufs=1))
        xbp = ctx.enter_context(tc.tile_pool(name="xbp", bufs=1))
        ew = ctx.enter_context(tc.tile_pool(name="ew", bufs=3))
        gpsum = ctx.enter_context(tc.tile_pool(name="gpsum", bufs=1, space="PSUM"))
        hpsum = ctx.enter_context(tc.tile_pool(name="hpsum", bufs=1, space="PSUM"))
        # manual PSUM double-buffer (static addresses; safe inside For_i)
        pbuf = [gpsum.tile([128, GC, G4, BLOC], f32, name=f"P{j}")
                for j in range(2)]

        # constants into SBUF
        wih_sb = consts.tile([128, 2, 4 * H], f8)
        nc.sync.dma_start(out=wih_sb[:, :, :], in_=wih[:, :, :])
        whh_sb = consts.tile([128, KC, 4 * H], f8)
        nc.sync.dma_start(out=whh_sb[:, :, :], in_=whh[:, :, :])
        wd_sb = consts.tile([128, KC, 1], f32)
        nc.sync.dma_start(out=wd_sb[:, :, :], in_=wd[:, :, :])

        # double-buffered by step parity to keep cross-step WAR hazards off
        # the critical path: at the start of step s, h(s-1) lives in
        # hbuf[s%2]; mul_h(s) writes h(s) into hbuf[(s+1)%2]. gcat packs
        # [ghat(2 chunks), c(2 chunks)] so one tensor_mul computes both
        # i*ghat and f*c; tanh(g)(s) writes gbuf[s%2][0:2] while add(s)
        # writes c(s) into gbuf[(s+1)%2][2:4].
        hbuf = [state.tile([128, KC, BLOC], f8, name=f"hT{j}") for j in range(2)]
        gbuf = [state.tile([128, 4, BLOC], f32, name=f"gcat{j}") for j in range(2)]
        # PACK10 state: slots [ghat0,ghat1, si0,si1, sf0,sf1, so0,so1, c0,c1]
        sbuf10 = [state.tile([128, 10, BLOC], f32, name=f"s10_{j}") for j in range(2)]
        for j in range(2):
            nc.vector.memset(hbuf[j][:, :, :], 0.0)
            nc.vector.memset(gbuf[j][:, :, :], 0.0)
            nc.vector.memset(sbuf10[j][:, :, :], 0.0)

        if not FORI:
            # whole input (already d-major / pair-interleaved)
            xTb = xbp.tile([128, ntc, 2, TC * BLOC], f8)
            nc.sync.dma_start(out=xTb[:, :, :, :], in_=x[:, :, :, :])
        else:
            assert ntc % 2 == 0, "FORI needs an even number of x chunks"

        def emit_group(xsrc, tau0, sbase, P):
            # one PSUM group: projection+bias for G4 steps, then the G4
            # recurrent steps. xsrc is a static [128, 2, TC*BLOC] view/tile;
            # sbase is the python step index (parity source) within the
            # unrolled region; P a static PSUM tile.
            xvw = xsrc[:, :, tau0 * BLOC:(tau0 + G4) * BLOC]
            gcs_per_bank = max(1, 512 // (G4 * BLOC))
            for gc in range(GC):
                # start=True zeroes the whole 2KB PSUM bank, so only the
                # first MM touching each bank may set it
                nc.tensor.matmul(
                    P[:, gc, :, :].rearrange("p t b -> p (t b)"),
                    lhsT=wih_sb[:, :, gc * 128:(gc + 1) * 128],
                    rhs=xvw,
                    start=(gc % gcs_per_bank == 0), stop=False,
                    skip_group_check=True,
                    perf_mode=DR,
                )
            for tt in range(G4):
                s = sbase + tt
                hT = hbuf[s % 2]
                hTn = hbuf[(s + 1) % 2]
                gcat = gbuf[s % 2]
                gcatn = gbuf[(s + 1) % 2]
                # recurrence: one DoubleRow MM per gate chunk (K=256)
                if NODR:
                    for gc in range(GC):
                        for kc in range(KC):
                            nc.tensor.matmul(
                                P[:, gc, tt, :],
                                lhsT=whh_sb[:, kc, gc * 128:(gc + 1) * 128],
                                rhs=hT[:, kc, :],
                                start=False, stop=(kc == KC - 1),
                                skip_group_check=True,
                            )
                else:
                    for gc in range(GC):
                        nc.tensor.matmul(
                            P[:, gc, tt, :],
                            lhsT=whh_sb[:, :, gc * 128:(gc + 1) * 128],
                            rhs=hT[:, :, :],
                            start=False, stop=True,
                            skip_group_check=True,
                            perf_mode=DR,
                        )
                abl = ABL.split(",")
                if PACK10:
                    # tanh tables; one strided mul computes [ghat,sf]*[si,c]
                    cur = sbuf10[s % 2]
                    nxt = sbuf10[(s + 1) % 2]
                    nc.scalar.activation(cur[:, 0:2, :], P[:, 0:2, tt, :], AF.Tanh)
                    nc.scalar.activation(cur[:, 2:8, :], P[:, 2:8, tt, :], AF.Sigmoid)
                    prod = ew.tile([128, 4, BLOC], f32, tag="prod")
                    # [ghat, sf] * [si, c] = slots {0,1,4,5} * {2,3,8,9}
                    in0 = cur[:, 0:6, :].rearrange(
                        "p (a b) x -> p a b x", a=3)[:, 0::2, :, :]
                    in1 = cur[:, 2:10, :].rearrange(
                        "p (a b) x -> p a b x", a=4)[:, 0::3, :, :]
                    nc.vector.tensor_mul(
                        prod[:, :, :].rearrange("p (a b) x -> p a b x", a=2),
                        in0, in1)
                    nc.vector.tensor_add(nxt[:, 8:10, :], prod[:, 0:2, :],
                                         prod[:, 2:4, :])
                    thc = ew.tile([128, 2, BLOC], f32, tag="thc")
                    nc.scalar.activation(thc[:, :, :], nxt[:, 8:10, :], AF.Tanh)
                    sb_ifo = cur  # head reads sigma(o) at [6:8]
                    nc.vector.tensor_mul(hTn[:, :, :], cur[:, 6:8, :], thc[:, :, :])
                elif SIGTRICK:
                    # sigmoid-only chain (g rows pre-scaled 2x on host):
                    #   s = sigmoid([2g, i, f, o])
                    #   ig = (2*s_g - 1) * s_i        (tanh(g) fused into mul)
                    #   fc = s_f * c
                    #   c' = ig + fc
                    #   h  = (2*sigmoid(2c') - 1) * s_o
                    from concourse.dve_ops import AFFINE_MUL_REDUCE
                    ewdt = f16 if F16EW else f32
                    sb_sig = ew.tile([128, 8, BLOC], ewdt, tag="sb_sig")
                    if SIGSPLIT:
                        nc.scalar.activation(sb_sig[:, 0:6, :], P[:, 0:6, tt, :],
                                             AF.Sigmoid)
                        nc.scalar.activation(sb_sig[:, 6:8, :], P[:, 6:8, tt, :],
                                             AF.Sigmoid)
                    else:
                        nc.scalar.activation(sb_sig[:, :, :], P[:, 0:8, tt, :],
                                             AF.Sigmoid)
                    prod = ew.tile([128, 4, BLOC], ewdt, tag="prod")
                    import concourse.mybir as _mb
                    if NOCUST:
                        ghat = ew.tile([128, 2, BLOC], f32, tag="ghat")
                        nc.vector.tensor_scalar(
                            ghat[:, :, :], sb_sig[:, 0:2, :], 2.0, -1.0,
                            _mb.AluOpType.mult, _mb.AluOpType.add)
                        nc.vector.tensor_mul(prod[:, 0:2, :], ghat[:, :, :],
                                             sb_sig[:, 2:4, :])
                    else:
                        nc.vector._custom_dve(
                            AFFINE_MUL_REDUCE, out=prod[:, 0:2, :],
                            in0=sb_sig[:, 0:2, :], in1=sb_sig[:, 2:4, :],
                            s0=2.0, s1=-1.0)
                    eng_fc = nc.gpsimd if POOLFC else nc.vector
                    eng_fc.tensor_mul(prod[:, 2:4, :], sb_sig[:, 4:6, :],
                                      gcat[:, 2:4, :])
                    nc.vector.tensor_add(gcatn[:, 2:4, :], prod[:, 0:2, :],
                                         prod[:, 2:4, :])
                    thc = ew.tile([128, 2, BLOC], ewdt, tag="thc")
                    nc.scalar.activation(thc[:, :, :], gcatn[:, 2:4, :],
                                         AF.Sigmoid, scale=2.0)
                    sb_ifo = sb_sig  # head reads sigma(o) at [6:8]
                    if NOCUST:
                        th2 = ew.tile([128, 2, BLOC], f32, tag="th2")
                        nc.vector.tensor_scalar(
                            th2[:, :, :], thc[:, :, :], 2.0, -1.0,
                            _mb.AluOpType.mult, _mb.AluOpType.add)
                        nc.vector.tensor_mul(hTn[:, :, :], th2[:, :, :],
                                             sb_sig[:, 6:8, :])
                    else:
                        nc.vector._custom_dve(
                            AFFINE_MUL_REDUCE, out=hTn[:, :, :],
                            in0=thc[:, :, :], in1=sb_sig[:, 6:8, :],
                            s0=2.0, s1=-1.0)
                else:
                    # elementwise cell update:
                    #   ghat = tanh(g); [i,f,o] = sigmoid(...)
                    #   prod = [i, f] * [ghat, c];  c = prod0 + prod1
                    #   h = o * tanh(c)
                    if "tg" not in abl:
                        nc.scalar.activation(gcat[:, 0:2, :], P[:, 0:2, tt, :], AF.Tanh)
                    sb_ifo = ew.tile([128, 6, BLOC], f32, tag="sb_ifo")
                    if "sif" not in abl:
                        if SIGSPLIT:
                            nc.scalar.activation(sb_ifo[:, 0:4, :], P[:, 2:6, tt, :], AF.Sigmoid)
                            nc.scalar.activation(sb_ifo[:, 4:6, :], P[:, 6:8, tt, :], AF.Sigmoid)
                        else:
                            nc.scalar.activation(sb_ifo[:, :, :], P[:, 2:8, tt, :], AF.Sigmoid)
                    prod = ew.tile([128, 4, BLOC], f32, tag="prod")
                    if "mul" not in abl:
                        nc.vector.tensor_mul(prod[:, :, :], sb_ifo[:, 0:4, :], gcat[:, :, :])
                    if "add" not in abl:
                        nc.vector.tensor_add(gcatn[:, 2:4, :], prod[:, 0:2, :], prod[:, 2:4, :])
                    thc = ew.tile([128, 2, BLOC], f32, tag="thc")
                    if "tc" not in abl:
                        nc.scalar.activation(thc[:, :, :], gcatn[:, 2:4, :], AF.Tanh)
                    if "mh" not in abl:
                        nc.vector.tensor_mul(hTn[:, :, :], sb_ifo[:, 4:6, :], thc[:, :, :])
            return sb_ifo, thc

        if FORI:
            # stage x chunks into static SBUF tiles via (dynamic-offset) DMA;
            # two chunks per iteration so buffer choice stays python-static
            xst = [xbp.tile([128, 2, TC * BLOC], f8, name=f"xst{j}")
                   for j in range(2)]
            gpc = TC // G4          # groups per chunk
            for _ in range(REPEAT):
                with tc.For_i(0, ntc, step=2) as ci_var:
                    nc.sync.dma_start(out=xst[0][:, :, :], in_=x[:, ci_var, :, :])
                    nc.sync.dma_start(out=xst[1][:, :, :],
                                      in_=x[:, ci_var + 1, :, :])
                    for half in range(2):
                        for gl in range(gpc):
                            g = half * gpc + gl
                            sb_ifo, thc = emit_group(
                                xst[half], gl * G4, g * G4, pbuf[g % 2])
        else:
            for gi in range(REPEAT * seq_len // G4):
                t0 = (gi * G4) % seq_len
                sb_ifo, thc = emit_group(
                    xTb[:, t0 // TC, :, :], t0 % TC, gi * G4, pbuf[gi % 2])

        # head: rebuild final h in f32 (avoid fp8 h), then
        # d = h @ w_d; probs = [sigmoid(d+bd), sigmoid(-d-bd)]
        hT32 = consts.tile([128, KC, BLOC], f32)
        if PACK10:
            nc.vector.tensor_mul(hT32[:, :, :], sb_ifo[:, 6:8, :], thc[:, :, :])
        elif SIGTRICK:
            if NOCUST:
                import concourse.mybir as _mb
                th2h = consts.tile([128, KC, BLOC], f32)
                nc.vector.tensor_scalar(
                    th2h[:, :, :], thc[:, :, :], 2.0, -1.0,
                    _mb.AluOpType.mult, _mb.AluOpType.add)
                nc.vector.tensor_mul(hT32[:, :, :], th2h[:, :, :],
                                     sb_ifo[:, 6:8, :])
            else:
                from concourse.dve_ops import AFFINE_MUL_REDUCE
                nc.vector._custom_dve(
                    AFFINE_MUL_REDUCE, out=hT32[:, :, :], in0=thc[:, :, :],
                    in1=sb_ifo[:, 6:8, :], s0=2.0, s1=-1.0)
        else:
            nc.vector.tensor_mul(hT32[:, :, :], sb_ifo[:, 4:6, :], thc[:, :, :])
        # head accumulator reuses a pbuf bank (PSUM may be full at G4=8)
        hps = pbuf[0][0:1, 0, 0, :]
        nc.tensor.matmul(hps[:, :], lhsT=wd_sb[:, 0, :], rhs=hT32[:, 0, :],
                         start=True, stop=False, skip_group_check=True)
        nc.tensor.matmul(hps[:, :], lhsT=wd_sb[:, 1, :], rhs=hT32[:, 1, :],
                         start=False, stop=True, skip_group_check=True)
        outsb = consts.tile([1, 2, BLOC], f32)
        bd_pos = consts.tile([1, 1], f32)
        bd_neg = consts.tile([1, 1], f32)
        nc.vector.memset(bd_pos[:, :], float(_cache["b_d"]))
        nc.vector.memset(bd_neg[:, :], -float(_cache["b_d"]))
        nc.scalar.activation(outsb[:, 0, :], hps[:, :], AF.Sigmoid,
                             bias=bd_pos[:, :], scale=1.0)
        nc.scalar.activation(outsb[:, 1, :], hps[:, :], AF.Sigmoid,
                             bias=bd_neg[:, :], scale=-1.0)
        nc.sync.dma_start(out=out[:, :, :], in_=outsb[:, :, :])

    nc.compile()
    return nc


def _prep_host(inputs, W_ih, W_hh, b_ih, b_hh, W_lin, b_lin):
    """Host-side weight preprocessing: gate permutation + transposed layouts."""
    import concourse.mybir as _mb
    f8np = _mb.dt.np(_mb.dt.float8e4)
    # PyTorch gate row order [i, f, g, o] (256 each) -> chunk order
    # [g0, g1, i0, i1, f0, f1, o0, o1] (128-row chunks)
    perm = np.concatenate([
        np.arange(512, 768),    # g
        np.arange(0, 256),      # i
        np.arange(256, 512),    # f
        np.arange(768, 1024),   # o
    ])

    Wih_p = np.ascontiguousarray(W_ih[perm]).astype(np.float32)  # [1024, 128]
    Whh_p = np.ascontiguousarray(W_hh[perm]).astype(np.float32)  # [1024, 256]
    b_p = (b_ih + b_hh)[perm].astype(np.float32)        # [1024]
    if SIGTRICK and not PACK10:
        # tanh(g) = 2*sigmoid(2g) - 1: fold the 2x into the g-gate rows
        # (exact power-of-2 scale, no extra fp8 rounding error)
        Wih_p[0:256] *= 2.0
        Whh_p[0:256] *= 2.0
        b_p[0:256] *= 2.0

    # projection lhsT with bias in pair-half 1: [128(d), 2, 1024]
    wih_host = np.zeros((128, 2, 4 * H), np.float32)
    wih_host[:, 0, :] = Wih_p.T
    wih_host[0, 1, :] = b_p
    wih_host = wih_host.astype(f8np)

    # recurrence lhsT: [128(k within chunk), KC, 1024]
    whh_host = np.ascontiguousarray(
        Whh_p.T.reshape(KC, 128, 4 * H).transpose(1, 0, 2)
    ).astype(f8np)

    w_d = (W_lin[0] - W_lin[1]).astype(np.float32)                  # [256]
    wd_host = np.ascontiguousarray(
        w_d.reshape(KC, 128).T.reshape(128, KC, 1)).astype(np.float32)
    b_d = float(b_lin[0] - b_lin[1])

    # x: [256, T, 128] f32 -> [128(d), T/TC, 2(pair), TC, B] fp8 with
    # pair-half 1 = (d==0) indicator (per-core batch slice + reshape to
    # [128, T/TC, 2, TC*BLOC] happens in kernel())
    x8 = inputs.astype(f8np)                                        # [256, T, 128]
    xT = np.transpose(x8, (2, 1, 0))                                # [128, T, 256]
    ntc = T // TC
    x_host = np.zeros((128, ntc, 2, TC, B), f8np)
    x_host[:, :, 0, :, :] = xT.reshape(128, ntc, TC, B)
    x_host[0, :, 1, :, :] = f8np(1.0)
    return x_host, wih_host, whh_host, wd_host, b_d


def kernel(inputs, W_ih, W_hh, b_ih, b_hh, W_lin, b_lin):
    from concourse.bass_utils import run_bass_kernel_spmd

    inputs = np.asarray(inputs, dtype=np.float32)
    x_host, wih_h, whh_h, wd_h, b_d = _prep_host(
        np.asarray(inputs), np.asarray(W_ih), np.asarray(W_hh),
        np.asarray(b_ih), np.asarray(b_hh), np.asarray(W_lin), np.asarray(b_lin))
    if _cache.get("b_d") != b_d or "nc" not in _cache:
        _cache["b_d"] = b_d
        _cache["nc"] = _build_program(T)
    nc = _cache["nc"]

    in_maps = []
    for j in range(NCORES):
        xj = np.ascontiguousarray(x_host[:, :, :, :, j * BLOC:(j + 1) * BLOC])
        in_maps.append({
            "x": xj.reshape(128, T // TC, 2, TC * BLOC),
            "wih": wih_h, "whh": whh_h, "wd": wd_h,
        })

    res = run_bass_kernel_spmd(nc, in_maps, core_ids=list(range(NCORES)))
    _cache["last_result"] = res
    out = np.concatenate(
        [np.asarray(r["out"])[0].T for r in res.results], axis=0)
    return np.ascontiguousarray(out).astype(np.float32)


# revision 45
# speedup vs baseline: 1.1910x; 1.1317x over previous
"""Trainium2 Bass kernel for BasicLSTM (nn_BasicLSTM_16320875724833).

Problem: inputs [256, 1024, 128] f32; LSTM(H=256) over T=1024 steps, then
linear [256->2] + softmax on the final hidden state. Output [256, 2] f32.

Strategy (8 cores, data-parallel over batch, 32 rows/core):
  - All state kept "transposed" (feature-major): hT/cT are [128p, 2, 32].
  - fp8e4 DoubleRow matmuls everywhere on the recurrence path:
      * recurrence: one MM per gate chunk (K=256 folded into the pair dim),
        8 MMs/step, rhs = hT [128, 2, 32] fp8.
      * input projection: x is pre-transposed on host to [128, 2, T*BLOC]
        fp8 where pair-half 1 is a (d==0) indicator channel; the projection
        weights' pair-half 1 carries the bias row, so one DR MM per gate
        chunk computes W_ih^T x + (b_ih + b_hh) for 4 steps at once --
        no separate bias matmuls.
  - Gate chunk order (host-side row permutation of the PyTorch [i,f,g,o]
    layout): [g0,g1, i0,i1, f0,f1, o0,o1] so tanh(g) fires after 2 MMs and
    sigmoid(i,f) after 6.
  - Head: softmax over 2 classes == [sigmoid(d), sigmoid(-d)] with
    d = h @ (W_lin[0]-W_lin[1]) + (b_lin[0]-b_lin[1]); h rebuilt in f32
    from the final step's sigmoid(o) and tanh(c) so fp8 h never feeds it.
"""

import numpy as np

# ---- problem constants (hardcoded; kernel.py must be self-contained) ----
B, T, D, H = 256, 1024, 128, 256
NCORES = 8
BLOC = B // NCORES          # 32 batch rows per core
GC = 8                      # gate chunks of 128 (4H = 1024)
KC = 2                      # hidden chunks of 128 (H = 256)
import os as _os
G4 = int(_os.environ.get("K_G4", "4"))   # timesteps per PSUM group
TC = 64                     # time chunk for x layout (keeps the DoubleRow
                            # pair-dim stride = TC*BLOC = 2048 within the
                            # ISA's 16-bit AP step field)

import os
SIGSPLIT = os.environ.get("K_SIGSPLIT", "1") == "1"  # (i,f) + (o) split
GBUFS = int(os.environ.get("K_GBUFS", "2"))          # gates PSUM pool buffers
SIGTRICK = os.environ.get("K_SIG", "1") == "1"       # sigmoid-only EW chain
PACK10 = os.environ.get("K_PACK10", "0") == "1"      # tanh tables + 10-slot state
POOLFC = os.environ.get("K_POOL", "0") == "1"        # f*c multiply on Pool
NOCUST = os.environ.get("K_NOCUST", "0") == "1"      # avoid custom DVE ops
NODR = os.environ.get("K_NODR", "0") == "1"          # plain (non-DoubleRow) rec MMs
FORI = os.environ.get("K_FORI", "1") == "1"          # hardware loop over TC chunks
F16EW = os.environ.get("K_F16EW", "0") == "1"        # fp16 elementwise temporaries
ABL = os.environ.get("K_ABL", "")    # timing-only ablations, comma-separated
REPEAT = 1                  # timing-only: run the recurrence REPEAT times

_cache = {}


def _build_program(seq_len=T):
    import concourse.bass as bass
    import concourse.mybir as mybir
    from concourse import bacc
    from concourse.tile import TileContext
    from contextlib import ExitStack

    f16 = mybir.dt.float16
    f32 = mybir.dt.float32
    f8 = mybir.dt.float8e4
    AF = mybir.ActivationFunctionType
    DR = mybir.MatmulPerfMode.DoubleRow

    nc = bacc.Bacc(None, target_bir_lowering=False)

    # x pre-transposed+interleaved on host:
    # [128(d), seq/TC, 2(pair), TC*BLOC] fp8; pair-half 1 = (d==0) indicator
    ntc = (seq_len + TC - 1) // TC
    x = nc.dram_tensor("x", [128, ntc, 2, TC * BLOC], f8, kind="ExternalInput")
    # projection weights with bias folded into pair-half 1: [128, 2, 4H] fp8
    wih = nc.dram_tensor("wih", [128, 2, 4 * H], f8, kind="ExternalInput")
    # recurrence weights: [128(k), KC(pair), 4H] fp8
    whh = nc.dram_tensor("whh", [128, KC, 4 * H], f8, kind="ExternalInput")
    wd = nc.dram_tensor("wd", [128, KC, 1], f32, kind="ExternalInput")
    out = nc.dram_tensor("out", [1, 2, BLOC], f32, kind="ExternalOutput")

    with ExitStack() as ctx:
        tc = ctx.enter_context(TileContext(nc))
        consts = ctx.enter_context(tc.tile_pool(name="consts", bufs=1))
        state = ctx.enter_context(tc.tile_pool(name="state", bufs=1))
        xbp = ctx.enter_context(tc.tile_pool(name="xbp", bufs=1))
        ew = ctx.enter_context(tc.tile_pool(name="ew", bufs=3))
        gpsum = ctx.enter_context(tc.tile_pool(name="gpsum", bufs=1, space="PSUM"))
        hpsum = ctx.enter_context(tc.tile_pool(name="hpsum", bufs=1, space="PSUM"))
        # manual PSUM double-buffer (static addresses; safe inside For_i)
        pbuf = [gpsum.tile([128, GC, G4, BLOC], f32, name=f"P{j}")
                for j in range(2)]

        # constants into SBUF
        wih_sb = consts.tile([128, 2, 4 * H], f8)
        nc.sync.dma_start(out=wih_sb[:, :, :], in_=wih[:, :, :])
        whh_sb = consts.tile([128, KC, 4 * H], f8)
        nc.sync.dma_start(out=whh_sb[:, :, :], in_=whh[:, :, :])
        wd_sb = consts.tile([128, KC, 1], f32)
        nc.sync.dma_start(out=wd_sb[:, :, :], in_=wd[:, :, :])

        # double-buffered by step parity to keep cross-step WAR hazards off
        # the critical path: at the start of step s, h(s-1) lives in
        # hbuf[s%2]; mul_h(s) writes h(s) into hbuf[(s+1)%2]. gcat packs
        # [ghat(2 chunks), c(2 chunks)] so one tensor_mul computes both
        # i*ghat and f*c; tanh(g)(s) writes gbuf[s%2][0:2] while add(s)
        # writes c(s) into gbuf[(s+1)%2][2:4].
        hbuf = [state.tile([128, KC, BLOC], f8, name=f"hT{j}") for j in range(2)]
        gbuf = [state.tile([128, 4, BLOC], f32, name=f"gcat{j}") for j in range(2)]
        # PACK10 state: slots [ghat0,ghat1, si0,si1, sf0,sf1, so0,so1, c0,c1]
        sbuf10 = [state.tile([128, 10, BLOC], f32, name=f"s10_{j}") for j in range(2)]
        for j in range(2):
            nc.vector.memset(hbuf[j][:, :, :], 0.0)
            nc.vector.memset(gbuf[j][:, :, :], 0.0)
            if PACK10:
                nc.vector.memset(sbuf10[j][:, :, :], 0.0)

        if not FORI:
            # whole input (already d-major / pair-interleaved)
            xTb = xbp.tile([128, ntc, 2, TC * BLOC], f8)
            nc.sync.dma_start(out=xTb[:, :, :, :], in_=x[:, :, :, :])
        else:
            assert ntc % 2 == 0, "FORI needs an even number of x chunks"

        def emit_group(xsrc, tau0, sbase, P):
            # one PSUM group: projection+bias for G4 steps, then the G4
            # recurrent steps. xsrc is a static [128, 2, TC*BLOC] view/tile;
            # sbase is the python step index (parity source) within the
            # unrolled region; P a static PSUM tile.
            xvw = xsrc[:, :, tau0 * BLOC:(tau0 + G4) * BLOC]
            gcs_per_bank = max(1, 512 // (G4 * BLOC))
            for gc in range(GC):
                # start=True zeroes the whole 2KB PSUM bank, so only the
                # first MM touching each bank may set it
                nc.tensor.matmul(
                    P[:, gc, :, :].rearrange("p t b -> p (t b)"),
                    lhsT=wih_sb[:, :, gc * 128:(gc + 1) * 128],
                    rhs=xvw,
                    start=(gc % gcs_per_bank == 0), stop=False,
                    skip_group_check=True,
                    perf_mode=DR,
                )
            for tt in range(G4):
                s = sbase + tt
                hT = hbuf[s % 2]
                hTn = hbuf[(s + 1) % 2]
                gcat = gbuf[s % 2]
                gcatn = gbuf[(s + 1) % 2]
                # recurrence: one DoubleRow MM per gate chunk (K=256)
                if NODR:
                    for gc in range(GC):
                        for kc in range(KC):
                            nc.tensor.matmul(
                                P[:, gc, tt, :],
                                lhsT=whh_sb[:, kc, gc * 128:(gc + 1) * 128],
                                rhs=hT[:, kc, :],
                                start=False, stop=(kc == KC - 1),
                                skip_group_check=True,
                            )
                else:
                    for gc in range(GC):
                        nc.tensor.matmul(
                            P[:, gc, tt, :],
                            lhsT=whh_sb[:, :, gc * 128:(gc + 1) * 128],
                            rhs=hT[:, :, :],
                            start=False, stop=True,
                            skip_group_check=True,
                            perf_mode=DR,
                        )
                abl = ABL.split(",")
                if PACK10:
                    # tanh tables; one strided mul computes [ghat,sf]*[si,c]
                    cur = sbuf10[s % 2]
                    nxt = sbuf10[(s + 1) % 2]
                    nc.scalar.activation(cur[:, 0:2, :], P[:, 0:2, tt, :], AF.Tanh)
                    nc.scalar.activation(cur[:, 2:8, :], P[:, 2:8, tt, :], AF.Sigmoid)
                    prod = ew.tile([128, 4, BLOC], f32, tag="prod")
                    # [ghat, sf] * [si, c] = slots {0,1,4,5} * {2,3,8,9}
                    in0 = cur[:, 0:6, :].rearrange(
                        "p (a b) x -> p a b x", a=3)[:, 0::2, :, :]
                    in1 = cur[:, 2:10, :].rearrange(
                        "p (a b) x -> p a b x", a=4)[:, 0::3, :, :]
                    nc.vector.tensor_mul(
                        prod[:, :, :].rearrange("p (a b) x -> p a b x", a=2),
                        in0, in1)
                    nc.vector.tensor_add(nxt[:, 8:10, :], prod[:, 0:2, :],
                                         prod[:, 2:4, :])
                    thc = ew.tile([128, 2, BLOC], f32, tag="thc")
                    nc.scalar.activation(thc[:, :, :], nxt[:, 8:10, :], AF.Tanh)
                    sb_ifo = cur  # head reads sigma(o) at [6:8]
                    nc.vector.tensor_mul(hTn[:, :, :], cur[:, 6:8, :], thc[:, :, :])
                elif SIGTRICK:
                    # sigmoid-only chain (g rows pre-scaled 2x on host):
                    #   s = sigmoid([2g, i, f, o])
                    #   ig = (2*s_g - 1) * s_i        (tanh(g) fused into mul)
                    #   fc = s_f * c
                    #   c' = ig + fc
                    #   h  = (2*sigmoid(2c') - 1) * s_o
                    from concourse.dve_ops import AFFINE_MUL_REDUCE
                    ewdt = f16 if F16EW else f32
                    sb_sig = ew.tile([128, 8, BLOC], ewdt, tag="sb_sig")
                    if SIGSPLIT:
                        nc.scalar.activation(sb_sig[:, 0:6, :], P[:, 0:6, tt, :],
                                             AF.Sigmoid)
                        nc.scalar.activation(sb_sig[:, 6:8, :], P[:, 6:8, tt, :],
                                             AF.Sigmoid)
                    else:
                        nc.scalar.activation(sb_sig[:, :, :], P[:, 0:8, tt, :],
                                             AF.Sigmoid)
                    prod = ew.tile([128, 4, BLOC], ewdt, tag="prod")
                    import concourse.mybir as _mb
                    if NOCUST:
                        ghat = ew.tile([128, 2, BLOC], f32, tag="ghat")
                        nc.vector.tensor_scalar(
                            ghat[:, :, :], sb_sig[:, 0:2, :], 2.0, -1.0,
                            _mb.AluOpType.mult, _mb.AluOpType.add)
                        nc.vector.tensor_mul(prod[:, 0:2, :], ghat[:, :, :],
                                             sb_sig[:, 2:4, :])
                    else:
                        nc.vector._custom_dve(
                            AFFINE_MUL_REDUCE, out=prod[:, 0:2, :],
                            in0=sb_sig[:, 0:2, :], in1=sb_sig[:, 2:4, :],
                            s0=2.0, s1=-1.0)
                    eng_fc = nc.gpsimd if POOLFC else nc.vector
                    eng_fc.tensor_mul(prod[:, 2:4, :], sb_sig[:, 4:6, :],
                                      gcat[:, 2:4, :])
                    nc.vector.tensor_add(gcatn[:, 2:4, :], prod[:, 0:2, :],
                                         prod[:, 2:4, :])
                    thc = ew.tile([128, 2, BLOC], ewdt, tag="thc")
                    nc.scalar.activation(thc[:, :, :], gcatn[:, 2:4, :],
                                         AF.Sigmoid, scale=2.0)
                    sb_ifo = sb_sig  # head reads sigma(o) at [6:8]
                    if NOCUST:
                        th2 = ew.tile([128, 2, BLOC], f32, tag="th2")
                        nc.vector.tensor_scalar(
                            th2[:, :, :], thc[:, :, :], 2.0, -1.0,
                            _mb.AluOpType.mult, _mb.AluOpType.add)
                        nc.vector.tensor_mul(hTn[:, :, :], th2[:, :, :],
                                             sb_sig[:, 6:8, :])
                    else:
                        nc.vector._custom_dve(
                            AFFINE_MUL_REDUCE, out=hTn[:, :, :],
                            in0=thc[:, :, :], in1=sb_sig[:, 6:8, :],
                            s0=2.0, s1=-1.0)
                else:
                    # elementwise cell update:
                    #   ghat = tanh(g); [i,f,o] = sigmoid(...)
                    #   prod = [i, f] * [ghat, c];  c = prod0 + prod1
                    #   h = o * tanh(c)
                    if "tg" not in abl:
                        nc.scalar.activation(gcat[:, 0:2, :], P[:, 0:2, tt, :], AF.Tanh)
                    sb_ifo = ew.tile([128, 6, BLOC], f32, tag="sb_ifo")
                    if "sif" not in abl:
                        if SIGSPLIT:
                            nc.scalar.activation(sb_ifo[:, 0:4, :], P[:, 2:6, tt, :], AF.Sigmoid)
                            nc.scalar.activation(sb_ifo[:, 4:6, :], P[:, 6:8, tt, :], AF.Sigmoid)
                        else:
                            nc.scalar.activation(sb_ifo[:, :, :], P[:, 2:8, tt, :], AF.Sigmoid)
                    prod = ew.tile([128, 4, BLOC], f32, tag="prod")
                    if "mul" not in abl:
                        nc.vector.tensor_mul(prod[:, :, :], sb_ifo[:, 0:4, :], gcat[:, :, :])
                    if "add" not in abl:
                        nc.vector.tensor_add(gcatn[:, 2:4, :], prod[:, 0:2, :], prod[:, 2:4, :])
                    thc = ew.tile([128, 2, BLOC], f32, tag="thc")
                    if "tc" not in abl:
                        nc.scalar.activation(thc[:, :, :], gcatn[:, 2:4, :], AF.Tanh)
                    if "mh" not in abl:
                        nc.vector.tensor_mul(hTn[:, :, :], sb_ifo[:, 4:6, :], thc[:, :, :])
            return sb_ifo, thc

        if FORI:
            # stage x chunks into static SBUF tiles via (dynamic-offset) DMA;
            # two chunks per iteration so buffer choice stays python-static
            xst = [xbp.tile([128, 2, TC * BLOC], f8, name=f"xst{j}")
                   for j in range(2)]
            gpc = TC // G4          # groups per chunk
            for _ in range(REPEAT):
                with tc.For_i(0, ntc, step=2) as ci_var:
                    nc.sync.dma_start(out=xst[0][:, :, :], in_=x[:, ci_var, :, :])
                    nc.sync.dma_start(out=xst[1][:, :, :],
                                      in_=x[:, ci_var + 1, :, :])
                    for half in range(2):
                        for gl in range(gpc):
                            g = half * gpc + gl
                            sb_ifo, thc = emit_group(
                                xst[half], gl * G4, g * G4, pbuf[g % 2])
        else:
            for gi in range(REPEAT * seq_len // G4):
                t0 = (gi * G4) % seq_len
                sb_ifo, thc = emit_group(
                    xTb[:, t0 // TC, :, :], t0 % TC, gi * G4, pbuf[gi % 2])

        # head: rebuild final h in f32 (avoid fp8 h), then
        # d = h @ w_d; probs = [sigmoid(d+bd), sigmoid(-d-bd)]
        hT32 = consts.tile([128, KC, BLOC], f32)
        if PACK10:
            nc.vector.tensor_mul(hT32[:, :, :], sb_ifo[:, 6:8, :], thc[:, :, :])
        elif SIGTRICK:
            if NOCUST:
                import concourse.mybir as _mb
                th2h = consts.tile([128, KC, BLOC], f32)
                nc.vector.tensor_scalar(
                    th2h[:, :, :], thc[:, :, :], 2.0, -1.0,
                    _mb.AluOpType.mult, _mb.AluOpType.add)
                nc.vector.tensor_mul(hT32[:, :, :], th2h[:, :, :],
                                     sb_ifo[:, 6:8, :])
            else:
                from concourse.dve_ops import AFFINE_MUL_REDUCE
                nc.vector._custom_dve(
                    AFFINE_MUL_REDUCE, out=hT32[:, :, :], in0=thc[:, :, :],
                    in1=sb_ifo[:, 6:8, :], s0=2.0, s1=-1.0)
        else:
            nc.vector.tensor_mul(hT32[:, :, :], sb_ifo[:, 4:6, :], thc[:, :, :])
        # head accumulator reuses a pbuf bank (PSUM may be full at G4=8)
        hps = pbuf[0][0:1, 0, 0, :]
        nc.tensor.matmul(hps[:, :], lhsT=wd_sb[:, 0, :], rhs=hT32[:, 0, :],
                         start=True, stop=False, skip_group_check=True)
        nc.tensor.matmul(hps[:, :], lhsT=wd_sb[:, 1, :], rhs=hT32[:, 1, :],
                         start=False, stop=True, skip_group_check=True)
        outsb = consts.tile([1, 2, BLOC], f32)
        bd_pos = consts.tile([1, 1], f32)
        bd_neg = consts.tile([1, 1], f32)
        nc.vector.memset(bd_pos[:, :], float(_cache["b_d"]))
        nc.vector.memset(bd_neg[:, :], -float(_cache["b_d"]))
        nc.scalar.activation(outsb[:, 0, :], hps[:, :], AF.Sigmoid,
                             bias=bd_pos[:, :], scale=1.0)
        nc.scalar.activation(outsb[:, 1, :], hps[:, :], AF.Sigmoid,
                             bias=bd_neg[:, :], scale=-1.0)
        nc.sync.dma_start(out=out[:, :, :], in_=outsb[:, :, :])

    nc.compile()
    return nc


def _prep_host(inputs, W_ih, W_hh, b_ih, b_hh, W_lin, b_lin):
    """Host-side weight preprocessing: gate permutation + transposed layouts."""
    import concourse.mybir as _mb
    f8np = _mb.dt.np(_mb.dt.float8e4)
    # PyTorch gate row order [i, f, g, o] (256 each) -> chunk order
    # [g0, g1, i0, i1, f0, f1, o0, o1] (128-row chunks)
    perm = np.concatenate([
        np.arange(512, 768),    # g
        np.arange(0, 256),      # i
        np.arange(256, 512),    # f
        np.arange(768, 1024),   # o
    ])

    Wih_p = np.ascontiguousarray(W_ih[perm]).astype(np.float32)  # [1024, 128]
    Whh_p = np.ascontiguousarray(W_hh[perm]).astype(np.float32)  # [1024, 256]
    b_p = (b_ih + b_hh)[perm].astype(np.float32)        # [1024]
    if SIGTRICK and not PACK10:
        # tanh(g) = 2*sigmoid(2g) - 1: fold the 2x into the g-gate rows
        # (exact power-of-2 scale, no extra fp8 rounding error)
        Wih_p[0:256] *= 2.0
        Whh_p[0:256] *= 2.0
        b_p[0:256] *= 2.0

    # projection lhsT with bias in pair-half 1: [128(d), 2, 1024]
    wih_host = np.zeros((128, 2, 4 * H), np.float32)
    wih_host[:, 0, :] = Wih_p.T
    wih_host[0, 1, :] = b_p
    wih_host = wih_host.astype(f8np)

    # recurrence lhsT: [128(k within chunk), KC, 1024]
    whh_host = np.ascontiguousarray(
        Whh_p.T.reshape(KC, 128, 4 * H).transpose(1, 0, 2)
    ).astype(f8np)

    w_d = (W_lin[0] - W_lin[1]).astype(np.float32)                  # [256]
    wd_host = np.ascontiguousarray(
        w_d.reshape(KC, 128).T.reshape(128, KC, 1)).astype(np.float32)
    b_d = float(b_lin[0] - b_lin[1])

    # x: [256, T, 128] f32 -> [128(d), T/TC, 2(pair), TC, B] fp8 with
    # pair-half 1 = (d==0) indicator (per-core batch slice + reshape to
    # [128, T/TC, 2, TC*BLOC] happens in kernel())
    x8 = inputs.astype(f8np)                                        # [256, T, 128]
    xT = np.transpose(x8, (2, 1, 0))                                # [128, T, 256]
    ntc = T // TC
    x_host = np.zeros((128, ntc, 2, TC, B), f8np)
    x_host[:, :, 0, :, :] = xT.reshape(128, ntc, TC, B)
    x_host[0, :, 1, :, :] = f8np(1.0)
    return x_host, wih_host, whh_host, wd_host, b_d


def kernel(inputs, W_ih, W_hh, b_ih, b_hh, W_lin, b_lin):
    from concourse.bass_utils import run_bass_kernel_spmd

    inputs = np.asarray(inputs, dtype=np.float32)
    x_host, wih_h, whh_h, wd_h, b_d = _prep_host(
        np.asarray(inputs), np.asarray(W_ih), np.asarray(W_hh),
        np.asarray(b_ih), np.asarray(b_hh), np.asarray(W_lin), np.asarray(b_lin))
    if _cache.get("b_d") != b_d or "nc" not in _cache:
        _cache["b_d"] = b_d
        _cache["nc"] = _build_program(T)
    nc = _cache["nc"]

    in_maps = []
    for j in range(NCORES):
        xj = np.ascontiguousarray(x_host[:, :, :, :, j * BLOC:(j + 1) * BLOC])
        in_maps.append({
            "x": xj.reshape(128, T // TC, 2, TC * BLOC),
            "wih": wih_h, "whh": whh_h, "wd": wd_h,
        })

    res = run_bass_kernel_spmd(nc, in_maps, core_ids=list(range(NCORES)))
    _cache["last_result"] = res
    out = np.concatenate(
        [np.asarray(r["out"])[0].T for r in res.results], axis=0)
    return np.ascontiguousarray(out).astype(np.float32)
